# revision 1
# baseline (speedup 1.0000x reference)
"""Trainium2 Bass kernel for CRF Viterbi decode (nn_CRF).

Problem (hardcoded): x[64, 512, 1024] @ kernel[1024, 128] + bias -> logits
[B, T, U]; boundary energies added on first/last timestep; Viterbi decode
with transition matrix chain_kernel[128, 128]; returns tags as float32.

Strategy
--------
Data-parallel over 8 NeuronCores: 8 batch elements per core.

Device (per core):
  1. logits matmul: x^T (pre-transposed on host to [D, (t, b)] layout)
     against kernel tiles, accumulated in PSUM over K=1024, evacuated by
     the scalar engine (bias fused) into an SBUF-resident logitsT[u, (t,b)].
  2. Viterbi forward scan (511 serial steps), batches split into 2 groups
     of 4 that pipeline across engines. Per step and group:
       - PE broadcasts v rows into PSUM (K=1 matmuls against a ones row)
         and accumulates trans[i, j] on top via an identity-tiled matmul
         (exact fp32: each PSUM element sees v[b,i] + trans[i,j] as a
         single fp32 add).
       - DVE does a segmented reduce-max over i -> max values [u, 4].
       - DVE adds logit_t -> new v, written into an SBUF vhist tile.
       - PE transposes the new v column block; ACT evacuates it to SBUF
         as rows for the next step's broadcast matmuls.
     Only max VALUES are kept (no argmax on device) - backpointers are
     reconstructed exactly on the host from vhist, since fp32 add/max
     here are bit-exact reproducible.
  3. vhist is DMAd out chunk-by-chunk as the scan progresses.

Host: shard/pre-transpose inputs, run SPMD on cores 0-7, then backtrace:
  tag_t = argmax_i(v_t[b, i] + trans[i, tag_{t+1}]) - bit-identical to the
  backpointers the device forward pass implies.
"""

import os

import numpy as np

import concourse.bass as bass
import concourse.mybir as mybir
from concourse.tile import TileContext
from concourse.bass_utils import run_bass_kernel_spmd

F32 = mybir.dt.float32

# Problem constants
B, T, D, U = 64, 512, 1024, 128
NCORES = 8
BL = B // NCORES           # batches per core (8)
NG = 2                     # batch pipeline groups
GB = BL // NG              # batches per group (4)

last_results = None        # BassKernelResults of the most recent kernel() run


def split_multi_waits(nc):
    """The walrus build in this container encodes at most ONE sync wait per
    compute/DMA instruction ("Too many sync wait commands" otherwise). Hoist
    all but the last wait of any multi-wait instruction onto standalone
    same-engine EventSemaphore ops placed immediately before it (engine
    queues execute in order, so semantics are preserved)."""
    for f in nc.m.functions:
        for blk in f.blocks:
            new_insts = []
            changed = False
            for inst in blk.instructions:
                si = inst.sync_info
                if si is not None and len(si.on_wait) > 1:
                    waits = list(si.on_wait)
                    for k, w in enumerate(waits[:-1]):
                        new_insts.append(mybir.InstEventSemaphore(
                            name=f"{inst.name}-sw{k}",
                            engine=inst.engine,
                            ins=[], outs=[],
                            sync_info=mybir.SyncInfo(on_wait=[w], on_update=[]),
                        ))
                    inst.sync_info = mybir.SyncInfo(
                        on_wait=[waits[-1]], on_update=list(si.on_update))
                    changed = True
                new_insts.append(inst)
            if changed:
                blk.instructions = new_insts
    return nc


def build_program(t_steps=T, d_dim=D, split_waits=True, scan_reps=1,
                  loop_reps=None, warm=0):
    nt = t_steps * BL                       # columns in (t, b) layout
    ch = min(512, nt)                       # DMA/matmul chunk width
    nch = nt // ch
    kblocks = d_dim // 128

    nc = bass.Bass(trn_type="TRN2")

    xdt = nc.dram_tensor("xdt", [d_dim, nt], F32, kind="ExternalInput")
    ker = nc.dram_tensor("ker", [d_dim, U], F32, kind="ExternalInput")
    translhs = nc.dram_tensor("translhs", [U, U], F32, kind="ExternalInput")
    ident = nc.dram_tensor("ident", [U, U], F32, kind="ExternalInput")
    lbv = nc.dram_tensor("lbv", [U, 1], F32, kind="ExternalInput")
    rbv = nc.dram_tensor("rbv", [U, 1], F32, kind="ExternalInput")
    biasrow = nc.dram_tensor("biasrow", [1, U], F32, kind="ExternalInput")
    onesrow = nc.dram_tensor("onesrow", [1, 512], F32, kind="ExternalInput")
    vout = nc.dram_tensor("vout", [U, nt], F32, kind="ExternalOutput")

    with TileContext(nc) as tc:
        with (
            tc.tile_pool(name="const", bufs=1) as cpool,
            tc.tile_pool(name="xp", bufs=10) as xpool,
            tc.tile_pool(name="big", bufs=1) as bigpool,
            tc.tile_pool(name="mx", bufs=3) as mxpool,
            tc.tile_pool(name="mmps", bufs=2, space="PSUM") as mmpool,
            tc.tile_pool(name="scps", bufs=2, space="PSUM") as scpool,
            tc.tile_pool(name="wmps", bufs=1, space="PSUM") as wmpool,
        ):
            # ---- constants into SBUF ----
            ker_sb = []
            for kb in range(kblocks):
                kt = cpool.tile([128, U], F32, tag=f"ker{kb}")
                nc.sync.dma_start(out=kt[:, :], in_=ker[kb * 128:(kb + 1) * 128, :])
                ker_sb.append(kt)
            trans_sb = cpool.tile([U, U], F32, tag="trans")
            nc.sync.dma_start(out=trans_sb[:, :], in_=translhs[:, :])
            ident_sb = cpool.tile([U, U], F32, tag="ident")
            nc.sync.dma_start(out=ident_sb[:, :], in_=ident[:, :])
            lb_sb = cpool.tile([U, 1], F32, tag="lb")
            nc.sync.dma_start(out=lb_sb[:, :], in_=lbv[:, :])
            rb_sb = cpool.tile([U, 1], F32, tag="rb")
            nc.sync.dma_start(out=rb_sb[:, :], in_=rbv[:, :])
            biasrow_sb = cpool.tile([1, U], F32, tag="biasrow")
            nc.sync.dma_start(out=biasrow_sb[:, :], in_=biasrow[:, :])
            onesrow_sb = cpool.tile([1, 512], F32, tag="onesrow")
            nc.sync.dma_start(out=onesrow_sb[:, :], in_=onesrow[:, :])

            logitsT = bigpool.tile([U, nt], F32, tag="logitsT")
            # per-group v history (decoupled so the two batch-group
            # pipelines never serialize on tile dependency tracking);
            # group g columns: t * GB + bb
            vh = [bigpool.tile([U, nt // NG], F32, tag=f"vh{g}",
                                name=f"vh{g}")
                  for g in range(NG)]

            # ---- phase 1: logits = kernel.T @ x (+bias) ----
            for c in range(nch):
                ps = mmpool.tile([128, ch], F32, tag="mm")
                for kb in range(kblocks):
                    xt = xpool.tile([128, ch], F32, tag="x")
                    nc.sync.dma_start(
                        out=xt[:, :],
                        in_=xdt[kb * 128:(kb + 1) * 128, c * ch:(c + 1) * ch],
                    )
                    nc.tensor.matmul(
                        out=ps[:, :], lhsT=ker_sb[kb][:, :], rhs=xt[:, :],
                        start=(kb == 0), stop=False,
                    )
                nc.tensor.matmul(
                    out=ps[:, :], lhsT=biasrow_sb[0:1, :],
                    rhs=onesrow_sb[0:1, 0:ch], start=False, stop=True,
                )
                nc.scalar.copy(
                    out=logitsT[:, c * ch:(c + 1) * ch], in_=ps[:, :],
                )

            # right boundary folded into the last timestep's logits
            nc.vector.tensor_scalar_add(
                out=logitsT[:, (t_steps - 1) * BL:],
                in0=logitsT[:, (t_steps - 1) * BL:],
                scalar1=rb_sb[:, 0:1],
            )
            # ---- phase 2: Viterbi forward scan ----
            # v broadcast across partitions: matmul with the previous v
            # column as stationary operand, broadcast along its free dim
            # (step-0 AP), against an identity moving operand:
            #   out[p, i] = sum_k v[k] * I[k, i] = v[i]  for every p.
            # scan_reps > 1 repeats the whole scan (for differential
            # wall-clock timing); results are identical each rep.
            steps_per_chunk = ch // BL
            gch = steps_per_chunk * GB          # per-group chunk width
            import contextlib
            rep_ctx = (tc.For_i(0, loop_reps, 1) if loop_reps
                       else contextlib.nullcontext())
            with rep_ctx:
             for _rep in range(scan_reps):
              # v_0 = logits_0 + left boundary
              for g in range(NG):
                nc.vector.tensor_scalar_add(
                    out=vh[g][:, 0:GB], in0=logitsT[:, g * GB:(g + 1) * GB],
                    scalar1=lb_sb[:, 0:1],
                )
              for t in range(1, t_steps):
                  for g in range(NG):
                      lcols0 = t * BL + g * GB    # logitsT columns
                      vcols0 = t * GB             # vh[g] columns
                      pcol0 = (t - 1) * GB
                      sc = scpool.tile([128, GB * U], F32, tag=f"sc{g}")
                      for bb in range(GB):
                          vcol = vh[g][:, pcol0 + bb:pcol0 + bb + 1]
                          nc.tensor.matmul(
                              out=sc[:, bb * U:(bb + 1) * U],
                              lhsT=vcol.broadcast_to([U, U]), rhs=ident_sb[:, :],
                              start=(bb == 0), stop=False, skip_group_check=True,
                              is_transpose=True,
                          )
                      for bb in range(GB):
                          nc.tensor.matmul(
                              out=sc[:, bb * U:(bb + 1) * U],
                              lhsT=trans_sb[:, :], rhs=ident_sb[:, :],
                              start=False, stop=(bb == GB - 1),
                              skip_group_check=True, is_transpose=True,
                          )
                      # optional HAM-warming filler: keeps the PE p-state
                      # hot across the per-step stall waiting for v(t)
                      for _w in range(warm):
                          wt = wmpool.tile([U, U], F32, tag="warm")
                          nc.tensor.matmul(
                              out=wt[:, :], lhsT=ident_sb[:, :],
                              rhs=ident_sb[:, :], start=True, stop=True,
                              skip_group_check=True, is_transpose=True,
                          )
                      mx = mxpool.tile([U, GB], F32, tag=f"mx{g}")
                      nc.vector.tensor_reduce(
                          out=mx[:, :],
                          in_=sc[:, :].rearrange("p (b i) -> p b i", i=U),
                          axis=mybir.AxisListType.X, op=mybir.AluOpType.max,
                      )
                      # logit add on the otherwise-idle ACT engine, one
                      # column at a time (bias = per-partition logitsT col);
                      # lets each next-step broadcast start as soon as its
                      # own column is written
                      for bb in range(GB):
                          nc.scalar.activation(
                              out=vh[g][:, vcols0 + bb:vcols0 + bb + 1],
                              in_=mx[:, bb:bb + 1],
                              func=mybir.ActivationFunctionType.Identity,
                              bias=logitsT[:, lcols0 + bb:lcols0 + bb + 1],
                          )
                  if (t + 1) % steps_per_chunk == 0:
                      c = (t + 1) // steps_per_chunk - 1
                      for g in range(NG):
                          nc.sync.dma_start(
                              out=vout[:, g * (nt // NG) + c * gch:
                                       g * (nt // NG) + (c + 1) * gch],
                              in_=vh[g][:, c * gch:(c + 1) * gch],
                          )
    return split_multi_waits(nc) if split_waits else nc


def make_in_map(x_core, ker, bias, trans, lb, rb, t_steps=T, d_dim=D):
    """x_core: [BL, t_steps, d_dim] float32."""
    nt = t_steps * BL
    xdt = np.ascontiguousarray(x_core.transpose(2, 1, 0)).reshape(d_dim, nt)
    return {
        "xdt": xdt.astype(np.float32),
        "ker": np.ascontiguousarray(ker, dtype=np.float32),
        "biasrow": np.ascontiguousarray(bias, dtype=np.float32).reshape(1, U),
        "onesrow": np.ones((1, 512), dtype=np.float32),
        "translhs": np.ascontiguousarray(trans, dtype=np.float32),
        "ident": np.eye(U, dtype=np.float32),
        "lbv": np.ascontiguousarray(lb, dtype=np.float32).reshape(U, 1),
        "rbv": np.ascontiguousarray(rb, dtype=np.float32).reshape(U, 1),
    }


def backtrace(v, trans):
    """v: [b, t, u] forward max values; trans: [u, u]. Returns int tags [b, t]."""
    nb, nt, nu = v.shape
    tags = np.zeros((nb, nt), dtype=np.int64)
    cur = np.argmax(v[:, -1, :], axis=1)
    tags[:, -1] = cur
    for t in range(nt - 2, -1, -1):
        scores = v[:, t, :] + trans[:, cur].T     # fp32, same as device order
        cur = np.argmax(scores, axis=1)
        tags[:, t] = cur
    return tags


def vout_to_v(vout_core, t_steps=T):
    """vout [U, (g, t, bb)] -> v [BL, t, U] with b = g * GB + bb."""
    v = vout_core.reshape(U, NG, t_steps, GB)     # [u, g, t, bb]
    return np.ascontiguousarray(v.transpose(1, 3, 2, 0).reshape(BL, t_steps, U))


def kernel(x, kernel, bias, chain_kernel, left_boundary, right_boundary):
    x = np.asarray(x, dtype=np.float32)
    ker = np.asarray(kernel, dtype=np.float32)
    bias = np.asarray(bias, dtype=np.float32)
    trans = np.asarray(chain_kernel, dtype=np.float32)
    lb = np.asarray(left_boundary, dtype=np.float32)
    rb = np.asarray(right_boundary, dtype=np.float32)

    nc = build_program()
    in_maps = [
        make_in_map(x[c * BL:(c + 1) * BL], ker, bias, trans, lb, rb)
        for c in range(NCORES)
    ]
    kwargs = {}
    if os.environ.get("CRF_TRACE"):
        kwargs = {"trace": True, "tmpdir": os.environ.get("CRF_TRACE_DIR") or None}
    res = run_bass_kernel_spmd(nc, in_maps, core_ids=list(range(NCORES)), **kwargs)
    global last_results
    last_results = res
    v = np.concatenate(
        [vout_to_v(np.asarray(r["vout"])) for r in res.results], axis=0)
    tags = backtrace(v, trans)
    return tags.astype(np.float32)



# revision 2
# speedup vs baseline: 3.2892x; 3.2892x over previous
"""Trainium2 Bass kernel for CRF Viterbi decode (nn_CRF) — V2.

Problem (hardcoded): x[64, 512, 1024] @ kernel[1024, 128] + bias -> logits
[B, T, U]; boundary energies on first/last timestep; Viterbi decode with
transition matrix chain_kernel[128, 128]; returns tags as float32.

V2 changes vs baseline:
- trans-add done with ONE fp32r matmul per group (rhs = identity repeated 4x
  along free -> N=512 -> 1 cycle/row on PE) instead of 4 fp32 transpose-mode
  matmuls (2 cycles/row each). PE per-step cost drops ~2x.
- logit add: one batched DVE tensor_tensor add per group ([128, 4]) instead
  of 4 scalar-engine activations.
- The effective transition matrix the PE applies (fp32r may round) is read
  back from the device once (tprime output) and used for the host backtrace,
  keeping forward values and backtrace bit-consistent.

Per step and group (4 batches):
  PE:  4 broadcast matmuls (stationary = v column broadcast, rhs = identity,
       fp32 is_transpose, exact) + 1 fp32r trans-add matmul -> PSUM scores
       sc[j, (bb, i)] = v_bb[i] + trans[i, j].
  DVE: segmented reduce-max over i -> mx[128, 4]; then v = mx + logit cols
       (one [128,4] tensor_tensor add) written to vh.
Host: backtrace from vh exactly as the device computed it (using tprime).
"""

import os

import numpy as np

import concourse.bass as bass
import concourse.mybir as mybir
from concourse.tile import TileContext
from concourse.bass_utils import run_bass_kernel_spmd

F32 = mybir.dt.float32
F32R = mybir.dt.float32r

# Problem constants
B, T, D, U = 64, 512, 1024, 128
NCORES = 8
BL = B // NCORES           # batches per core (8)
NG = 2                     # batch pipeline groups
GB = BL // NG              # batches per group (4)

last_results = None


def split_multi_waits(nc):
    """The walrus build in this container encodes at most ONE sync wait per
    compute/DMA instruction. Hoist all but the last wait of any multi-wait
    instruction onto standalone same-engine EventSemaphore ops placed
    immediately before it."""
    for f in nc.m.functions:
        for blk in f.blocks:
            new_insts = []
            changed = False
            for inst in blk.instructions:
                si = inst.sync_info
                if si is not None and len(si.on_wait) > 1:
                    waits = list(si.on_wait)
                    for k, w in enumerate(waits[:-1]):
                        new_insts.append(mybir.InstEventSemaphore(
                            name=f"{inst.name}-sw{k}",
                            engine=inst.engine,
                            ins=[], outs=[],
                            sync_info=mybir.SyncInfo(on_wait=[w], on_update=[]),
                        ))
                    inst.sync_info = mybir.SyncInfo(
                        on_wait=[waits[-1]], on_update=list(si.on_update))
                    changed = True
                new_insts.append(inst)
            if changed:
                blk.instructions = new_insts
    return nc


def build_program(t_steps=T, d_dim=D, split_waits=True, scan_reps=1,
                  loop_reps=None):
    nt = t_steps * BL                       # columns in (t, b) layout
    ch = min(512, nt)                       # DMA/matmul chunk width
    nch = nt // ch
    kblocks = d_dim // 128

    nc = bass.Bass(trn_type="TRN2")

    xdt = nc.dram_tensor("xdt", [d_dim, nt], F32, kind="ExternalInput")
    ker = nc.dram_tensor("ker", [d_dim, U], F32, kind="ExternalInput")
    translhs = nc.dram_tensor("translhs", [U, U], F32, kind="ExternalInput")
    ident = nc.dram_tensor("ident", [U, U], F32, kind="ExternalInput")
    lbv = nc.dram_tensor("lbv", [U, 1], F32, kind="ExternalInput")
    rbv = nc.dram_tensor("rbv", [U, 1], F32, kind="ExternalInput")
    biasrow = nc.dram_tensor("biasrow", [1, U], F32, kind="ExternalInput")
    onesrow = nc.dram_tensor("onesrow", [1, 512], F32, kind="ExternalInput")
    vout = nc.dram_tensor("vout", [U, nt], F32, kind="ExternalOutput")
    # effective transposed transition as the fp32r matmul computes it:
    # tprime[j, i] = trans'[i, j]
    tprime = nc.dram_tensor("tprime", [U, U], F32, kind="ExternalOutput")

    with TileContext(nc) as tc:
        with (
            tc.tile_pool(name="const", bufs=1) as cpool,
            tc.tile_pool(name="xp", bufs=10) as xpool,
            tc.tile_pool(name="big", bufs=1) as bigpool,
            tc.tile_pool(name="mx", bufs=3) as mxpool,
            tc.tile_pool(name="mmps", bufs=2, space="PSUM") as mmpool,
            tc.tile_pool(name="sc0ps", bufs=2, space="PSUM") as scpool0,
            tc.tile_pool(name="sc1ps", bufs=2, space="PSUM") as scpool1,
        ):
            scpools = [scpool0, scpool1]
            # ---- constants into SBUF ----
            ker_sb = []
            for kb in range(kblocks):
                kt = cpool.tile([128, U], F32, tag=f"ker{kb}")
                nc.sync.dma_start(out=kt[:, :], in_=ker[kb * 128:(kb + 1) * 128, :])
                ker_sb.append(kt)
            trans_sb = cpool.tile([U, U], F32, tag="trans")
            nc.sync.dma_start(out=trans_sb[:, :], in_=translhs[:, :])
            ident_sb = cpool.tile([U, U], F32, tag="ident")
            nc.sync.dma_start(out=ident_sb[:, :], in_=ident[:, :])
            lb_sb = cpool.tile([U, 1], F32, tag="lb")
            nc.sync.dma_start(out=lb_sb[:, :], in_=lbv[:, :])
            rb_sb = cpool.tile([U, 1], F32, tag="rb")
            nc.sync.dma_start(out=rb_sb[:, :], in_=rbv[:, :])
            biasrow_sb = cpool.tile([1, U], F32, tag="biasrow")
            nc.sync.dma_start(out=biasrow_sb[:, :], in_=biasrow[:, :])
            onesrow_sb = cpool.tile([1, 512], F32, tag="onesrow")
            nc.sync.dma_start(out=onesrow_sb[:, :], in_=onesrow[:, :])

            logitsT = bigpool.tile([U, nt], F32, tag="logitsT")
            vh = [bigpool.tile([U, nt // NG], F32, tag=f"vh{g}", name=f"vh{g}")
                  for g in range(NG)]

            # fp32r-rounded copies of the trans-add operands (the BIR verifier
            # requires fp32r matmul inputs to be produced rounded-to-fp32r)
            trans_r_sb = cpool.tile([U, U], F32R, tag="trans_r")
            nc.vector.tensor_copy(out=trans_r_sb[:, :], in_=trans_sb[:, :])
            ident_r_sb = cpool.tile([U, U], F32R, tag="ident_r")
            nc.vector.tensor_copy(out=ident_r_sb[:, :], in_=ident_sb[:, :])
            trans_r = trans_r_sb[:, :]
            identrep_r = (ident_r_sb[:, :]
                          .rearrange("p (x i) -> p x i", x=1)
                          .broadcast_to([U, GB, U]))

            # ---- tprime readback: what the fp32r matmul actually adds ----
            tp = mmpool.tile([128, GB * U], F32, tag="mm")
            nc.tensor.matmul(out=tp[:, :], lhsT=trans_r, rhs=identrep_r,
                             start=True, stop=True)
            tpsb = mxpool.tile([U, U], F32, tag="tpsb")
            nc.scalar.copy(out=tpsb[:, :], in_=tp[:, 0:U])
            nc.sync.dma_start(out=tprime[:, :], in_=tpsb[:, :])

            # ---- phase 1: logits = kernel.T @ x (+bias) ----
            for c in range(nch):
                ps = mmpool.tile([128, ch], F32, tag="mm")
                for kb in range(kblocks):
                    xt = xpool.tile([128, ch], F32, tag="x")
                    nc.sync.dma_start(
                        out=xt[:, :],
                        in_=xdt[kb * 128:(kb + 1) * 128, c * ch:(c + 1) * ch],
                    )
                    nc.tensor.matmul(
                        out=ps[:, :], lhsT=ker_sb[kb][:, :], rhs=xt[:, :],
                        start=(kb == 0), stop=False,
                    )
                nc.tensor.matmul(
                    out=ps[:, :], lhsT=biasrow_sb[0:1, :],
                    rhs=onesrow_sb[0:1, 0:ch], start=False, stop=True,
                )
                nc.scalar.copy(
                    out=logitsT[:, c * ch:(c + 1) * ch], in_=ps[:, :],
                )

            # right boundary folded into the last timestep's logits
            nc.vector.tensor_scalar_add(
                out=logitsT[:, (t_steps - 1) * BL:],
                in0=logitsT[:, (t_steps - 1) * BL:],
                scalar1=rb_sb[:, 0:1],
            )

            # ---- phase 2: Viterbi forward scan ----
            steps_per_chunk = ch // BL
            gch = steps_per_chunk * GB          # per-group chunk width
            import contextlib
            rep_ctx = (tc.For_i(0, loop_reps, 1) if loop_reps
                       else contextlib.nullcontext())
            with rep_ctx:
             for _rep in range(scan_reps):
              # v_0 = logits_0 + left boundary
              for g in range(NG):
                nc.vector.tensor_scalar_add(
                    out=vh[g][:, 0:GB], in0=logitsT[:, g * GB:(g + 1) * GB],
                    scalar1=lb_sb[:, 0:1],
                )
              for t in range(1, t_steps):
                  for g in range(NG):
                      lcols0 = t * BL + g * GB    # logitsT columns
                      vcols0 = t * GB             # vh[g] columns
                      pcol0 = (t - 1) * GB
                      sc = scpools[g].tile([128, GB * U], F32, tag=f"sc{g}")
                      # broadcast v columns: sc[j, (bb, i)] = v_bb[i]
                      for bb in range(GB):
                          vcol = vh[g][:, pcol0 + bb:pcol0 + bb + 1]
                          nc.tensor.matmul(
                              out=sc[:, bb * U:(bb + 1) * U],
                              lhsT=vcol.broadcast_to([U, U]), rhs=ident_sb[:, :],
                              start=(bb == 0), stop=False, skip_group_check=True,
                              is_transpose=True,
                          )
                      # += trans[i, j] for all 4 batches in one fp32r matmul
                      nc.tensor.matmul(
                          out=sc[:, :], lhsT=trans_r, rhs=identrep_r,
                          start=False, stop=True, skip_group_check=True,
                      )
                      mx = mxpool.tile([U, GB], F32, tag=f"mx{g}")
                      nc.vector.tensor_reduce(
                          out=mx[:, :],
                          in_=sc[:, :].rearrange("p (b i) -> p b i", i=U),
                          axis=mybir.AxisListType.X, op=mybir.AluOpType.max,
                      )
                      # v = mx + logits columns (batched, one DVE op)
                      nc.vector.tensor_tensor(
                          out=vh[g][:, vcols0:vcols0 + GB],
                          in0=mx[:, :],
                          in1=logitsT[:, lcols0:lcols0 + GB],
                          op=mybir.AluOpType.add,
                      )
                  if (t + 1) % steps_per_chunk == 0:
                      c = (t + 1) // steps_per_chunk - 1
                      for g in range(NG):
                          nc.sync.dma_start(
                              out=vout[:, g * (nt // NG) + c * gch:
                                       g * (nt // NG) + (c + 1) * gch],
                              in_=vh[g][:, c * gch:(c + 1) * gch],
                          )
    return split_multi_waits(nc) if split_waits else nc


def make_in_map(x_core, ker, bias, trans, lb, rb, t_steps=T, d_dim=D):
    """x_core: [BL, t_steps, d_dim] float32."""
    nt = t_steps * BL
    xdt = np.ascontiguousarray(x_core.transpose(2, 1, 0)).reshape(d_dim, nt)
    return {
        "xdt": xdt.astype(np.float32),
        "ker": np.ascontiguousarray(ker, dtype=np.float32),
        "biasrow": np.ascontiguousarray(bias, dtype=np.float32).reshape(1, U),
        "onesrow": np.ones((1, 512), dtype=np.float32),
        "translhs": np.ascontiguousarray(trans, dtype=np.float32),
        "ident": np.eye(U, dtype=np.float32),
        "lbv": np.ascontiguousarray(lb, dtype=np.float32).reshape(U, 1),
        "rbv": np.ascontiguousarray(rb, dtype=np.float32).reshape(U, 1),
    }


def backtrace(v, trans):
    """v: [b, t, u] forward max values; trans: [u, u]. Returns int tags [b, t]."""
    nb, nt, nu = v.shape
    tags = np.zeros((nb, nt), dtype=np.int64)
    cur = np.argmax(v[:, -1, :], axis=1)
    tags[:, -1] = cur
    for t in range(nt - 2, -1, -1):
        scores = v[:, t, :] + trans[:, cur].T     # fp32, same as device order
        cur = np.argmax(scores, axis=1)
        tags[:, t] = cur
    return tags


def vout_to_v(vout_core, t_steps=T):
    """vout [U, (g, t, bb)] -> v [BL, t, U] with b = g * GB + bb."""
    v = vout_core.reshape(U, NG, t_steps, GB)     # [u, g, t, bb]
    return np.ascontiguousarray(v.transpose(1, 3, 2, 0).reshape(BL, t_steps, U))


def kernel(x, kernel, bias, chain_kernel, left_boundary, right_boundary):
    x = np.asarray(x, dtype=np.float32)
    ker = np.asarray(kernel, dtype=np.float32)
    bias = np.asarray(bias, dtype=np.float32)
    trans = np.asarray(chain_kernel, dtype=np.float32)
    lb = np.asarray(left_boundary, dtype=np.float32)
    rb = np.asarray(right_boundary, dtype=np.float32)

    nc = build_program()
    in_maps = [
        make_in_map(x[c * BL:(c + 1) * BL], ker, bias, trans, lb, rb)
        for c in range(NCORES)
    ]
    kwargs = {}
    if os.environ.get("CRF_TRACE"):
        kwargs = {"trace": True, "tmpdir": os.environ.get("CRF_TRACE_DIR") or None}
    res = run_bass_kernel_spmd(nc, in_maps, core_ids=list(range(NCORES)), **kwargs)
    global last_results
    last_results = res
    v = np.concatenate(
        [vout_to_v(np.asarray(r["vout"])) for r in res.results], axis=0)
    # the transition the device actually added (fp32r-rounded), for a
    # bit-consistent backtrace
    tprime = np.asarray(res.results[0]["tprime"])          # [j, i] = trans'[i, j]
    trans_eff = np.ascontiguousarray(tprime.T)
    tags = backtrace(v, trans_eff)
    return tags.astype(np.float32)


# revision 3
# speedup vs baseline: 4.9254x; 1.4974x over previous
"""Trainium2 Bass kernel for CRF Viterbi decode (nn_CRF) — V2.

Problem (hardcoded): x[64, 512, 1024] @ kernel[1024, 128] + bias -> logits
[B, T, U]; boundary energies on first/last timestep; Viterbi decode with
transition matrix chain_kernel[128, 128]; returns tags as float32.

V2 changes vs baseline:
- trans-add done with ONE fp32r matmul per group (rhs = identity repeated 4x
  along free -> N=512 -> 1 cycle/row on PE) instead of 4 fp32 transpose-mode
  matmuls (2 cycles/row each). PE per-step cost drops ~2x.
- logit add: one batched DVE tensor_tensor add per group ([128, 4]) instead
  of 4 scalar-engine activations.
- The effective transition matrix the PE applies (fp32r may round) is read
  back from the device once (tprime output) and used for the host backtrace,
  keeping forward values and backtrace bit-consistent.

Per step and group (4 batches):
  PE:  4 broadcast matmuls (stationary = v column broadcast, rhs = identity,
       fp32 is_transpose, exact) + 1 fp32r trans-add matmul -> PSUM scores
       sc[j, (bb, i)] = v_bb[i] + trans[i, j].
  DVE: segmented reduce-max over i -> mx[128, 4]; then v = mx + logit cols
       (one [128,4] tensor_tensor add) written to vh.
Host: backtrace from vh exactly as the device computed it (using tprime).
"""

import os

import numpy as np

import concourse.bass as bass
import concourse.mybir as mybir
from concourse.tile import TileContext
from concourse.bass_utils import run_bass_kernel_spmd

F32 = mybir.dt.float32
F32R = mybir.dt.float32r

# Problem constants
B, T, D, U = 64, 512, 1024, 128
NCORES = 8
BL = B // NCORES           # batches per core (8)
NG = 2                     # batch pipeline groups
GB = BL // NG              # batches per group (4)

last_results = None


def split_multi_waits(nc):
    """The walrus build in this container encodes at most ONE sync wait per
    compute/DMA instruction. Hoist all but the last wait of any multi-wait
    instruction onto standalone same-engine EventSemaphore ops placed
    immediately before it."""
    for f in nc.m.functions:
        for blk in f.blocks:
            new_insts = []
            changed = False
            for inst in blk.instructions:
                si = inst.sync_info
                if si is not None and len(si.on_wait) > 1:
                    waits = list(si.on_wait)
                    for k, w in enumerate(waits[:-1]):
                        new_insts.append(mybir.InstEventSemaphore(
                            name=f"{inst.name}-sw{k}",
                            engine=inst.engine,
                            ins=[], outs=[],
                            sync_info=mybir.SyncInfo(on_wait=[w], on_update=[]),
                        ))
                    inst.sync_info = mybir.SyncInfo(
                        on_wait=[waits[-1]], on_update=list(si.on_update))
                    changed = True
                new_insts.append(inst)
            if changed:
                blk.instructions = new_insts
    return nc


VARIANTS = {
    "v2": {},
    "noladd": {"ladd": False},          # timing-only: skip logit add
    "halfred": {"red_frac": 0.5},       # timing-only: reduce half the i-range
    "dblpe": {"extra_pe": 4},           # 4 extra bcast matmuls per group-step
    "warm2": {"warm": 2},               # 2 filler fp32r matmuls per group-step
    "ng4": {"ng": 4},                   # 4 pipeline groups of 2 batches
    "ladd_act": {"ladd": "act"},        # logit add on scalar engine per column
    "pf": {"prefire": True},            # trans matmul first (off the chain)
    "pf_rs2": {"prefire": True, "redsplit": 2},
    "pf_rs4": {"prefire": True, "redsplit": 4},
    "pf_ng4": {"prefire": True, "ng": 4},
    "pf_ng4_b2": {"prefire": True, "ng": 4, "scbufs": 2},
    "pf_ng4_gp": {"prefire": True, "ng": 4, "ladd": "gpsimd"},
    "pf_ng4_b2_gp": {"prefire": True, "ng": 4, "scbufs": 2, "ladd": "gpsimd"},
    "pf_ng4_warm1": {"prefire": True, "ng": 4, "warm": 1},
    "pf_ng4_hr": {"prefire": True, "ng": 4, "red_frac": 0.5},
    "pf_ng4_npe": {"prefire": True, "ng": 4, "skip_bcast": True},
    "pf_ng4_b2_gp_rs2": {"prefire": True, "ng": 4, "scbufs": 2,
                         "ladd": "gpsimd", "redsplit": 2},
    "ship_mb8": {"prefire": True, "ng": 4, "scbufs": 2, "ladd": "gpsimd",
                 "mxbufs": 8},
    "ship_pair": {"prefire": True, "ng": 4, "scbufs": 2, "ladd": "gpsimd",
                  "mxbufs": 8, "pairred": True},
    "ship_mb16": {"prefire": True, "ng": 4, "scbufs": 2, "ladd": "gpsimd",
                  "mxbufs": 16},
    "ship_ttsplit": {"prefire": True, "ng": 4, "scbufs": 2, "ladd": "split",
                     "mxbufs": 8},
}

# shipping configuration: best measured variant
SHIP = {"prefire": True, "ng": 4, "scbufs": 2, "ladd": "gpsimd", "mxbufs": 8}


def build_program(t_steps=T, d_dim=D, split_waits=True, scan_reps=1,
                  loop_reps=None, ladd=True, red_frac=1.0, extra_pe=0,
                  warm=0, ng=NG, prefire=False, redsplit=1, scbufs=None,
                  skip_bcast=False, mxbufs=3, pairred=False):
    NGv = ng
    GBv = BL // NGv
    nt = t_steps * BL                       # columns in (t, b) layout
    ch = min(512, nt)                       # DMA/matmul chunk width
    nch = nt // ch
    kblocks = d_dim // 128
    redw = int(U * red_frac)                # reduce width per batch (timing exp)

    nc = bass.Bass(trn_type="TRN2")

    xdt = nc.dram_tensor("xdt", [d_dim, nt], F32, kind="ExternalInput")
    ker = nc.dram_tensor("ker", [d_dim, U], F32, kind="ExternalInput")
    translhs = nc.dram_tensor("translhs", [U, U], F32, kind="ExternalInput")
    ident = nc.dram_tensor("ident", [U, U], F32, kind="ExternalInput")
    lbv = nc.dram_tensor("lbv", [U, 1], F32, kind="ExternalInput")
    rbv = nc.dram_tensor("rbv", [U, 1], F32, kind="ExternalInput")
    biasrow = nc.dram_tensor("biasrow", [1, U], F32, kind="ExternalInput")
    onesrow = nc.dram_tensor("onesrow", [1, 512], F32, kind="ExternalInput")
    vout = nc.dram_tensor("vout", [U, nt], F32, kind="ExternalOutput")
    # effective transposed transition as the fp32r matmul computes it:
    # tprime[j, i] = trans'[i, j]
    tprime = nc.dram_tensor("tprime", [U, U], F32, kind="ExternalOutput")

    with TileContext(nc) as tc:
        if scbufs is None:
            scbufs = 2 if NGv <= 2 else 1
        scoped_mm = NGv * scbufs > 4        # free phase-1 PSUM before the scan
        import contextlib as _cl
        with _cl.ExitStack() as _st:
            cpool = _st.enter_context(tc.tile_pool(name="const", bufs=1))
            xpool = _st.enter_context(tc.tile_pool(name="xp", bufs=10))
            bigpool = _st.enter_context(tc.tile_pool(name="big", bufs=1))
            mxpool = _st.enter_context(tc.tile_pool(name="mx", bufs=mxbufs))
            mmctx = tc.tile_pool(name="mmps", bufs=2, space="PSUM")
            mmpool = mmctx.__enter__() if scoped_mm else _st.enter_context(mmctx)
            scpools = None

            def open_scpools():
                return [
                    _st.enter_context(
                        tc.tile_pool(name=f"sc{g}ps", bufs=scbufs,
                                     space="PSUM"))
                    for g in range(NGv)
                ]
            if not scoped_mm:
                scpools = open_scpools()
            # ---- constants into SBUF ----
            ker_sb = []
            for kb in range(kblocks):
                kt = cpool.tile([128, U], F32, tag=f"ker{kb}")
                nc.sync.dma_start(out=kt[:, :], in_=ker[kb * 128:(kb + 1) * 128, :])
                ker_sb.append(kt)
            trans_sb = cpool.tile([U, U], F32, tag="trans")
            nc.sync.dma_start(out=trans_sb[:, :], in_=translhs[:, :])
            ident_sb = cpool.tile([U, U], F32, tag="ident")
            nc.sync.dma_start(out=ident_sb[:, :], in_=ident[:, :])
            lb_sb = cpool.tile([U, 1], F32, tag="lb")
            nc.sync.dma_start(out=lb_sb[:, :], in_=lbv[:, :])
            rb_sb = cpool.tile([U, 1], F32, tag="rb")
            nc.sync.dma_start(out=rb_sb[:, :], in_=rbv[:, :])
            biasrow_sb = cpool.tile([1, U], F32, tag="biasrow")
            nc.sync.dma_start(out=biasrow_sb[:, :], in_=biasrow[:, :])
            onesrow_sb = cpool.tile([1, 512], F32, tag="onesrow")
            nc.sync.dma_start(out=onesrow_sb[:, :], in_=onesrow[:, :])

            logitsT = bigpool.tile([U, nt], F32, tag="logitsT")
            vh = [bigpool.tile([U, nt // NGv], F32, tag=f"vh{g}", name=f"vh{g}")
                  for g in range(NGv)]

            # fp32r-rounded copies of the trans-add operands (the BIR verifier
            # requires fp32r matmul inputs to be produced rounded-to-fp32r)
            trans_r_sb = cpool.tile([U, U], F32R, tag="trans_r")
            nc.vector.tensor_copy(out=trans_r_sb[:, :], in_=trans_sb[:, :])
            ident_r_sb = cpool.tile([U, U], F32R, tag="ident_r")
            nc.vector.tensor_copy(out=ident_r_sb[:, :], in_=ident_sb[:, :])
            trans_r = trans_r_sb[:, :]
            identrep_r = (ident_r_sb[:, :]
                          .rearrange("p (x i) -> p x i", x=1)
                          .broadcast_to([U, GBv, U]))

            # ---- tprime readback: what the fp32r matmul actually adds ----
            tp = mmpool.tile([128, GBv * U], F32, tag="mm")
            nc.tensor.matmul(out=tp[:, :], lhsT=trans_r, rhs=identrep_r,
                             start=True, stop=True)
            tpsb = mxpool.tile([U, U], F32, tag="tpsb")
            nc.scalar.copy(out=tpsb[:, :], in_=tp[:, 0:U])
            nc.sync.dma_start(out=tprime[:, :], in_=tpsb[:, :])

            # ---- phase 1: logits = kernel.T @ x (+bias) ----
            for c in range(nch):
                ps = mmpool.tile([128, ch], F32, tag="mm")
                for kb in range(kblocks):
                    xt = xpool.tile([128, ch], F32, tag="x")
                    nc.sync.dma_start(
                        out=xt[:, :],
                        in_=xdt[kb * 128:(kb + 1) * 128, c * ch:(c + 1) * ch],
                    )
                    nc.tensor.matmul(
                        out=ps[:, :], lhsT=ker_sb[kb][:, :], rhs=xt[:, :],
                        start=(kb == 0), stop=False,
                    )
                nc.tensor.matmul(
                    out=ps[:, :], lhsT=biasrow_sb[0:1, :],
                    rhs=onesrow_sb[0:1, 0:ch], start=False, stop=True,
                )
                nc.scalar.copy(
                    out=logitsT[:, c * ch:(c + 1) * ch], in_=ps[:, :],
                )

            # right boundary folded into the last timestep's logits
            nc.vector.tensor_scalar_add(
                out=logitsT[:, (t_steps - 1) * BL:],
                in0=logitsT[:, (t_steps - 1) * BL:],
                scalar1=rb_sb[:, 0:1],
            )

            if scoped_mm:
                mmctx.__exit__(None, None, None)
                scpools = open_scpools()

            # ---- phase 2: Viterbi forward scan ----
            steps_per_chunk = ch // BL
            gch = steps_per_chunk * GBv         # per-group chunk width
            import contextlib
            rep_ctx = (tc.For_i(0, loop_reps, 1) if loop_reps
                       else contextlib.nullcontext())
            with rep_ctx:
             for _rep in range(scan_reps):
              # v_0 = logits_0 + left boundary
              for g in range(NGv):
                nc.vector.tensor_scalar_add(
                    out=vh[g][:, 0:GBv], in0=logitsT[:, g * GBv:(g + 1) * GBv],
                    scalar1=lb_sb[:, 0:1],
                )
              for t in range(1, t_steps):
                  if pairred:
                      for pr in range(NGv // 2):
                          gA, gB = 2 * pr, 2 * pr + 1
                          sc = scpools[gA].tile([128, 2 * GBv * U], F32,
                                                tag=f"sc{gA}")
                          nc.tensor.matmul(
                              out=sc[:, :], lhsT=trans_r,
                              rhs=(ident_r_sb[:, :]
                                   .rearrange("p (x i) -> p x i", x=1)
                                   .broadcast_to([U, 2 * GBv, U])),
                              start=True, stop=False, skip_group_check=True,
                          )
                          for gi, g in enumerate((gA, gB)):
                              pcol0 = (t - 1) * GBv
                              for bb in range(GBv):
                                  vcol = vh[g][:, pcol0 + bb:pcol0 + bb + 1]
                                  nc.tensor.matmul(
                                      out=sc[:, (gi * GBv + bb) * U:
                                             (gi * GBv + bb + 1) * U],
                                      lhsT=vcol.broadcast_to([U, U]),
                                      rhs=ident_sb[:, :],
                                      start=False,
                                      stop=(gi == 1 and bb == GBv - 1),
                                      skip_group_check=True, is_transpose=True,
                                  )
                          mx = mxpool.tile([U, 2 * GBv], F32, tag=f"mx{gA}")
                          nc.vector.tensor_reduce(
                              out=mx[:, :],
                              in_=sc[:, :].rearrange("p (b i) -> p b i", i=U),
                              axis=mybir.AxisListType.X, op=mybir.AluOpType.max,
                          )
                          for gi, g in enumerate((gA, gB)):
                              lcols0 = t * BL + g * GBv
                              vcols0 = t * GBv
                              nc.gpsimd.tensor_tensor(
                                  out=vh[g][:, vcols0:vcols0 + GBv],
                                  in0=mx[:, gi * GBv:(gi + 1) * GBv],
                                  in1=logitsT[:, lcols0:lcols0 + GBv],
                                  op=mybir.AluOpType.add,
                              )
                      if (t + 1) % steps_per_chunk == 0:
                          c = (t + 1) // steps_per_chunk - 1
                          for g in range(NGv):
                              nc.sync.dma_start(
                                  out=vout[:, g * (nt // NGv) + c * gch:
                                           g * (nt // NGv) + (c + 1) * gch],
                                  in_=vh[g][:, c * gch:(c + 1) * gch],
                              )
                      continue
                  for g in range(NGv):
                      lcols0 = t * BL + g * GBv   # logitsT columns
                      vcols0 = t * GBv            # vh[g] columns
                      pcol0 = (t - 1) * GBv
                      sc = scpools[g].tile([128, GBv * U], F32, tag=f"sc{g}")
                      if prefire:
                          # trans matmul first: independent of v, fires as
                          # soon as the PSUM bank frees (off the chain)
                          nc.tensor.matmul(
                              out=sc[:, :], lhsT=trans_r, rhs=identrep_r,
                              start=True, stop=False, skip_group_check=True,
                          )
                      # broadcast v columns: sc[j, (bb, i)] += v_bb[i]
                      nbc = 1 if skip_bcast else GBv  # probe: 1 bcast only
                      for bb in range(nbc):
                          vcol = vh[g][:, pcol0 + bb:pcol0 + bb + 1]
                          nc.tensor.matmul(
                              out=sc[:, bb * U:(bb + 1) * U],
                              lhsT=vcol.broadcast_to([U, U]), rhs=ident_sb[:, :],
                              start=(bb == 0 and not prefire),
                              stop=(prefire and bb == nbc - 1),
                              skip_group_check=True,
                              is_transpose=True,
                          )
                      for _e in range(extra_pe):
                          vcol = vh[g][:, pcol0:pcol0 + 1]
                          nc.tensor.matmul(
                              out=sc[:, 0:U],
                              lhsT=vcol.broadcast_to([U, U]), rhs=ident_sb[:, :],
                              start=False, stop=False, skip_group_check=True,
                              is_transpose=True,
                          )
                      for _w in range(warm):
                          nc.tensor.matmul(
                              out=sc[:, :], lhsT=trans_r, rhs=identrep_r,
                              start=False, stop=False, skip_group_check=True,
                          )
                      if not prefire:
                          # += trans[i, j] for all batches in one fp32r matmul
                          nc.tensor.matmul(
                              out=sc[:, :], lhsT=trans_r, rhs=identrep_r,
                              start=False, stop=True, skip_group_check=True,
                          )
                      sb = GBv // redsplit          # batches per reduce op
                      mx = mxpool.tile([U, GBv], F32, tag=f"mx{g}")
                      for rs in range(redsplit):
                          nc.vector.tensor_reduce(
                              out=mx[:, rs * sb:(rs + 1) * sb],
                              in_=sc[:, rs * sb * U:rs * sb * U + sb * redw]
                                  .rearrange("p (b i) -> p b i", i=redw),
                              axis=mybir.AxisListType.X, op=mybir.AluOpType.max,
                          )
                          if ladd == "act":
                              for bb in range(rs * sb, (rs + 1) * sb):
                                  nc.scalar.activation(
                                      out=vh[g][:, vcols0 + bb:vcols0 + bb + 1],
                                      in_=mx[:, bb:bb + 1],
                                      func=mybir.ActivationFunctionType.Identity,
                                      bias=logitsT[:, lcols0 + bb:lcols0 + bb + 1],
                                  )
                          elif ladd == "gpsimd" or (ladd == "split" and g < NGv // 2):
                              nc.gpsimd.tensor_tensor(
                                  out=vh[g][:, vcols0 + rs * sb:
                                            vcols0 + (rs + 1) * sb],
                                  in0=mx[:, rs * sb:(rs + 1) * sb],
                                  in1=logitsT[:, lcols0 + rs * sb:
                                              lcols0 + (rs + 1) * sb],
                                  op=mybir.AluOpType.add,
                              )
                          elif ladd:
                              nc.vector.tensor_tensor(
                                  out=vh[g][:, vcols0 + rs * sb:
                                            vcols0 + (rs + 1) * sb],
                                  in0=mx[:, rs * sb:(rs + 1) * sb],
                                  in1=logitsT[:, lcols0 + rs * sb:
                                              lcols0 + (rs + 1) * sb],
                                  op=mybir.AluOpType.add,
                              )
                          else:
                              nc.vector.tensor_copy(
                                  out=vh[g][:, vcols0 + rs * sb:
                                            vcols0 + (rs + 1) * sb],
                                  in_=mx[:, rs * sb:(rs + 1) * sb],
                              )
                  if (t + 1) % steps_per_chunk == 0:
                      c = (t + 1) // steps_per_chunk - 1
                      for g in range(NGv):
                          nc.sync.dma_start(
                              out=vout[:, g * (nt // NGv) + c * gch:
                                       g * (nt // NGv) + (c + 1) * gch],
                              in_=vh[g][:, c * gch:(c + 1) * gch],
                          )
    return split_multi_waits(nc) if split_waits else nc


def make_in_map(x_core, ker, bias, trans, lb, rb, t_steps=T, d_dim=D):
    """x_core: [BL, t_steps, d_dim] float32."""
    nt = t_steps * BL
    xdt = np.ascontiguousarray(x_core.transpose(2, 1, 0)).reshape(d_dim, nt)
    return {
        "xdt": xdt.astype(np.float32),
        "ker": np.ascontiguousarray(ker, dtype=np.float32),
        "biasrow": np.ascontiguousarray(bias, dtype=np.float32).reshape(1, U),
        "onesrow": np.ones((1, 512), dtype=np.float32),
        "translhs": np.ascontiguousarray(trans, dtype=np.float32),
        "ident": np.eye(U, dtype=np.float32),
        "lbv": np.ascontiguousarray(lb, dtype=np.float32).reshape(U, 1),
        "rbv": np.ascontiguousarray(rb, dtype=np.float32).reshape(U, 1),
    }


def backtrace(v, trans):
    """v: [b, t, u] forward max values; trans: [u, u]. Returns int tags [b, t]."""
    nb, nt, nu = v.shape
    tags = np.zeros((nb, nt), dtype=np.int64)
    cur = np.argmax(v[:, -1, :], axis=1)
    tags[:, -1] = cur
    for t in range(nt - 2, -1, -1):
        scores = v[:, t, :] + trans[:, cur].T     # fp32, same as device order
        cur = np.argmax(scores, axis=1)
        tags[:, t] = cur
    return tags


def vout_to_v(vout_core, t_steps=T, ng=None):
    """vout [U, (g, t, bb)] -> v [BL, t, U] with b = g * gb + bb."""
    if ng is None:
        ng = SHIP.get("ng", NG)
    gb = BL // ng
    v = vout_core.reshape(U, ng, t_steps, gb)     # [u, g, t, bb]
    return np.ascontiguousarray(v.transpose(1, 3, 2, 0).reshape(BL, t_steps, U))


def kernel(x, kernel, bias, chain_kernel, left_boundary, right_boundary):
    x = np.asarray(x, dtype=np.float32)
    ker = np.asarray(kernel, dtype=np.float32)
    bias = np.asarray(bias, dtype=np.float32)
    trans = np.asarray(chain_kernel, dtype=np.float32)
    lb = np.asarray(left_boundary, dtype=np.float32)
    rb = np.asarray(right_boundary, dtype=np.float32)

    nc = build_program(**SHIP)
    in_maps = [
        make_in_map(x[c * BL:(c + 1) * BL], ker, bias, trans, lb, rb)
        for c in range(NCORES)
    ]
    kwargs = {}
    if os.environ.get("CRF_TRACE"):
        kwargs = {"trace": True, "tmpdir": os.environ.get("CRF_TRACE_DIR") or None}
    res = run_bass_kernel_spmd(nc, in_maps, core_ids=list(range(NCORES)), **kwargs)
    global last_results
    last_results = res
    v = np.concatenate(
        [vout_to_v(np.asarray(r["vout"])) for r in res.results], axis=0)
    # the transition the device actually added (fp32r-rounded), for a
    # bit-consistent backtrace
    tprime = np.asarray(res.results[0]["tprime"])          # [j, i] = trans'[i, j]
    trans_eff = np.ascontiguousarray(tprime.T)
    tags = backtrace(v, trans_eff)
    return tags.astype(np.float32)


# revision 8
# speedup vs baseline: 5.0400x; 1.0233x over previous
"""Trainium2 Bass kernel for CRF Viterbi decode (nn_CRF).

Problem (hardcoded): x[64, 512, 1024] @ kernel[1024, 128] + bias -> logits
[B, T, U]; boundary energies on first/last timestep; Viterbi decode with
transition matrix chain_kernel[128, 128]; returns tags as float32.

Data-parallel over 8 NeuronCores (8 batch elements per core). Per core the
Viterbi forward scan runs 511 serial steps; batches are split into ng=4
pipeline groups of 2 so the per-group serial chain (PE fill -> DVE reduce ->
logit add -> next fill) hides behind the other groups (SHIP config below).

Per step and group (2 batches):
  PE:  one fp32r trans-add matmul (prefired: start=True as soon as the PSUM
       bank frees, independent of v -> off the critical chain) + 2 broadcast
       matmuls (stationary = v column broadcast_to, rhs = identity, fp32
       is_transpose -> exact) accumulate PSUM scores
       sc[j, (bb, i)] = trans'[i, j] + v_bb[i].
  DVE: one segmented reduce-max over i -> mx[128, 2]   (the bottleneck:
       tensor_reduce is 1 elem/cycle/lane at 0.96 GHz from PSUM, and every
       other engine is structurally unable to help with max).
  GpSimd: v = mx + logit columns ([128, 2] tensor_tensor add) -> vh, keeping
       the add off the saturated DVE.
vh streams to HBM as the scan progresses.

Numerics: the fp32r matmul rounds the transition matrix (~2^-17); the
effective trans' is read back once via the tprime output and the HOST
backtrace uses it, so forward max values and backpointer reconstruction are
bit-consistent. Everything else (v broadcast, PSUM adds, reduce, logit adds)
is exact fp32; measured 4/32768 tags differ from the fp32 reference
(rel err 8.2e-3, well under the 2e-2 gate).

Host: shard/pre-transpose inputs, SPMD on cores 0-7, backtrace:
  tag_t = argmax_i(v_t[b, i] + trans'[i, tag_{t+1}]).
"""

import os

import numpy as np

import concourse.bass as bass
import concourse.mybir as mybir
from concourse.tile import TileContext
from concourse.bass_utils import run_bass_kernel_spmd

F32 = mybir.dt.float32
F32R = mybir.dt.float32r

# Problem constants
B, T, D, U = 64, 512, 1024, 128
NCORES = 8
BL = B // NCORES           # batches per core (8)
NG = 2                     # batch pipeline groups
GB = BL // NG              # batches per group (4)

last_results = None


def split_multi_waits(nc):
    """The walrus build in this container encodes at most ONE sync wait per
    compute/DMA instruction. Hoist all but the last wait of any multi-wait
    instruction onto standalone same-engine EventSemaphore ops placed
    immediately before it."""
    for f in nc.m.functions:
        for blk in f.blocks:
            new_insts = []
            changed = False
            for inst in blk.instructions:
                si = inst.sync_info
                if si is not None and len(si.on_wait) > 1:
                    waits = list(si.on_wait)
                    for k, w in enumerate(waits[:-1]):
                        new_insts.append(mybir.InstEventSemaphore(
                            name=f"{inst.name}-sw{k}",
                            engine=inst.engine,
                            ins=[], outs=[],
                            sync_info=mybir.SyncInfo(on_wait=[w], on_update=[]),
                        ))
                    inst.sync_info = mybir.SyncInfo(
                        on_wait=[waits[-1]], on_update=list(si.on_update))
                    changed = True
                new_insts.append(inst)
            if changed:
                blk.instructions = new_insts
    return nc


VARIANTS = {
    "v2": {},
    "noladd": {"ladd": False},          # timing-only: skip logit add
    "halfred": {"red_frac": 0.5},       # timing-only: reduce half the i-range
    "dblpe": {"extra_pe": 4},           # 4 extra bcast matmuls per group-step
    "warm2": {"warm": 2},               # 2 filler fp32r matmuls per group-step
    "ng4": {"ng": 4},                   # 4 pipeline groups of 2 batches
    "ladd_act": {"ladd": "act"},        # logit add on scalar engine per column
    "pf": {"prefire": True},            # trans matmul first (off the chain)
    "pf_rs2": {"prefire": True, "redsplit": 2},
    "pf_rs4": {"prefire": True, "redsplit": 4},
    "pf_ng4": {"prefire": True, "ng": 4},
    "pf_ng4_b2": {"prefire": True, "ng": 4, "scbufs": 2},
    "pf_ng4_gp": {"prefire": True, "ng": 4, "ladd": "gpsimd"},
    "pf_ng4_b2_gp": {"prefire": True, "ng": 4, "scbufs": 2, "ladd": "gpsimd"},
    "pf_ng4_warm1": {"prefire": True, "ng": 4, "warm": 1},
    "pf_ng4_hr": {"prefire": True, "ng": 4, "red_frac": 0.5},
    "pf_ng4_npe": {"prefire": True, "ng": 4, "skip_bcast": True},
    "pf_ng4_b2_gp_rs2": {"prefire": True, "ng": 4, "scbufs": 2,
                         "ladd": "gpsimd", "redsplit": 2},
    "ship_mb8": {"prefire": True, "ng": 4, "scbufs": 2, "ladd": "gpsimd",
                 "mxbufs": 8},
    "ship_pair": {"prefire": True, "ng": 4, "scbufs": 2, "ladd": "gpsimd",
                  "mxbufs": 8, "pairred": True},
    "ship_mb16": {"prefire": True, "ng": 4, "scbufs": 2, "ladd": "gpsimd",
                  "mxbufs": 16},
    "ship_ttsplit": {"prefire": True, "ng": 4, "scbufs": 2, "ladd": "split",
                     "mxbufs": 8},
}

# shipping configuration: best measured variant
SHIP = {"prefire": True, "ng": 4, "scbufs": 2, "ladd": "gpsimd", "mxbufs": 8}


def build_program(t_steps=T, d_dim=D, split_waits=True, scan_reps=1,
                  loop_reps=None, ladd=True, red_frac=1.0, extra_pe=0,
                  warm=0, ng=NG, prefire=False, redsplit=1, scbufs=None,
                  skip_bcast=False, mxbufs=3, pairred=False):
    NGv = ng
    GBv = BL // NGv
    nt = t_steps * BL                       # columns in (t, b) layout
    ch = min(512, nt)                       # DMA/matmul chunk width
    nch = nt // ch
    kblocks = d_dim // 128
    redw = int(U * red_frac)                # reduce width per batch (timing exp)

    nc = bass.Bass(trn_type="TRN2")

    xdt = nc.dram_tensor("xdt", [d_dim, nt], F32, kind="ExternalInput")
    ker = nc.dram_tensor("ker", [d_dim, U], F32, kind="ExternalInput")
    translhs = nc.dram_tensor("translhs", [U, U], F32, kind="ExternalInput")
    ident = nc.dram_tensor("ident", [U, U], F32, kind="ExternalInput")
    lbv = nc.dram_tensor("lbv", [U, 1], F32, kind="ExternalInput")
    rbv = nc.dram_tensor("rbv", [U, 1], F32, kind="ExternalInput")
    biasrow = nc.dram_tensor("biasrow", [1, U], F32, kind="ExternalInput")
    onesrow = nc.dram_tensor("onesrow", [1, 512], F32, kind="ExternalInput")
    vout = nc.dram_tensor("vout", [U, nt], F32, kind="ExternalOutput")
    # effective transposed transition as the fp32r matmul computes it:
    # tprime[j, i] = trans'[i, j]
    tprime = nc.dram_tensor("tprime", [U, U], F32, kind="ExternalOutput")

    with TileContext(nc) as tc:
        if scbufs is None:
            scbufs = 2 if NGv <= 2 else 1
        # PSUM budget: NGv pools x scbufs bufs x [128, sctile] fp32.
        # Phase 1 shares these pools (same tag+shape) so bank reuse between
        # the logits matmuls and the scan goes through normal pool WAR
        # rotation -- no cross-pool reuse races.
        # [128, 512] fp32 = 2KB/partition = one PSUM bank per buf
        sctile = 512 if NGv * scbufs <= 8 else GBv * U
        import contextlib as _cl
        with _cl.ExitStack() as _st:
            cpool = _st.enter_context(tc.tile_pool(name="const", bufs=1))
            xpool = _st.enter_context(tc.tile_pool(name="xp", bufs=10))
            bigpool = _st.enter_context(tc.tile_pool(name="big", bufs=1))
            mxpool = _st.enter_context(tc.tile_pool(name="mx", bufs=mxbufs))
            scpools = [
                _st.enter_context(
                    tc.tile_pool(name=f"sc{g}ps", bufs=scbufs, space="PSUM"))
                for g in range(NGv)
            ]
            def mm_tile(c):
                return scpools[c % NGv].tile([128, sctile], F32,
                                             tag=f"sc{c % NGv}",
                                             name=f"scps{c % NGv}")
            # ---- constants into SBUF ----
            ker_sb = []
            for kb in range(kblocks):
                kt = cpool.tile([128, U], F32, tag=f"ker{kb}")
                nc.sync.dma_start(out=kt[:, :], in_=ker[kb * 128:(kb + 1) * 128, :])
                ker_sb.append(kt)
            trans_sb = cpool.tile([U, U], F32, tag="trans")
            nc.sync.dma_start(out=trans_sb[:, :], in_=translhs[:, :])
            ident_sb = cpool.tile([U, U], F32, tag="ident")
            nc.sync.dma_start(out=ident_sb[:, :], in_=ident[:, :])
            lb_sb = cpool.tile([U, 1], F32, tag="lb")
            nc.sync.dma_start(out=lb_sb[:, :], in_=lbv[:, :])
            rb_sb = cpool.tile([U, 1], F32, tag="rb")
            nc.sync.dma_start(out=rb_sb[:, :], in_=rbv[:, :])
            biasrow_sb = cpool.tile([1, U], F32, tag="biasrow")
            nc.sync.dma_start(out=biasrow_sb[:, :], in_=biasrow[:, :])
            onesrow_sb = cpool.tile([1, 512], F32, tag="onesrow")
            nc.sync.dma_start(out=onesrow_sb[:, :], in_=onesrow[:, :])

            logitsT = bigpool.tile([U, nt], F32, tag="logitsT")
            vh = [bigpool.tile([U, nt // NGv], F32, tag=f"vh{g}", name=f"vh{g}")
                  for g in range(NGv)]

            # fp32r-rounded copies of the trans-add operands (the BIR verifier
            # requires fp32r matmul inputs to be produced rounded-to-fp32r)
            trans_r_sb = cpool.tile([U, U], F32R, tag="trans_r")
            nc.vector.tensor_copy(out=trans_r_sb[:, :], in_=trans_sb[:, :])
            ident_r_sb = cpool.tile([U, U], F32R, tag="ident_r")
            nc.vector.tensor_copy(out=ident_r_sb[:, :], in_=ident_sb[:, :])
            trans_r = trans_r_sb[:, :]
            identrep_r = (ident_r_sb[:, :]
                          .rearrange("p (x i) -> p x i", x=1)
                          .broadcast_to([U, GBv, U]))

            # ---- tprime readback: what the fp32r matmul actually adds ----
            tp = mm_tile(0)
            nc.tensor.matmul(out=tp[:, 0:GBv * U], lhsT=trans_r,
                             rhs=identrep_r, start=True, stop=True)
            tpsb = mxpool.tile([U, U], F32, tag="tpsb")
            nc.scalar.copy(out=tpsb[:, :], in_=tp[:, 0:U])
            nc.sync.dma_start(out=tprime[:, :], in_=tpsb[:, :])

            # ---- phase 1: logits = kernel.T @ x (+bias) ----
            for c in range(nch):
                ps = mm_tile(c)
                for kb in range(kblocks):
                    xt = xpool.tile([128, ch], F32, tag="x")
                    nc.sync.dma_start(
                        out=xt[:, :],
                        in_=xdt[kb * 128:(kb + 1) * 128, c * ch:(c + 1) * ch],
                    )
                    nc.tensor.matmul(
                        out=ps[:, :], lhsT=ker_sb[kb][:, :], rhs=xt[:, :],
                        start=(kb == 0), stop=False,
                    )
                nc.tensor.matmul(
                    out=ps[:, :], lhsT=biasrow_sb[0:1, :],
                    rhs=onesrow_sb[0:1, 0:ch], start=False, stop=True,
                )
                nc.scalar.copy(
                    out=logitsT[:, c * ch:(c + 1) * ch], in_=ps[:, :],
                )

            # right boundary folded into the last timestep's logits
            nc.vector.tensor_scalar_add(
                out=logitsT[:, (t_steps - 1) * BL:],
                in0=logitsT[:, (t_steps - 1) * BL:],
                scalar1=rb_sb[:, 0:1],
            )

            # ---- phase 2: Viterbi forward scan ----
            steps_per_chunk = ch // BL
            gch = steps_per_chunk * GBv         # per-group chunk width
            import contextlib
            rep_ctx = (tc.For_i(0, loop_reps, 1) if loop_reps
                       else contextlib.nullcontext())
            with rep_ctx:
             for _rep in range(scan_reps):
              # v_0 = logits_0 + left boundary
              for g in range(NGv):
                nc.vector.tensor_scalar_add(
                    out=vh[g][:, 0:GBv], in0=logitsT[:, g * GBv:(g + 1) * GBv],
                    scalar1=lb_sb[:, 0:1],
                )
              for t in range(1, t_steps):
                  if pairred:
                      for pr in range(NGv // 2):
                          gA, gB = 2 * pr, 2 * pr + 1
                          sc = scpools[gA].tile([128, 2 * GBv * U], F32,
                                                tag=f"sc{gA}")
                          nc.tensor.matmul(
                              out=sc[:, :], lhsT=trans_r,
                              rhs=(ident_r_sb[:, :]
                                   .rearrange("p (x i) -> p x i", x=1)
                                   .broadcast_to([U, 2 * GBv, U])),
                              start=True, stop=False, skip_group_check=True,
                          )
                          for gi, g in enumerate((gA, gB)):
                              pcol0 = (t - 1) * GBv
                              for bb in range(GBv):
                                  vcol = vh[g][:, pcol0 + bb:pcol0 + bb + 1]
                                  nc.tensor.matmul(
                                      out=sc[:, (gi * GBv + bb) * U:
                                             (gi * GBv + bb + 1) * U],
                                      lhsT=vcol.broadcast_to([U, U]),
                                      rhs=ident_sb[:, :],
                                      start=False,
                                      stop=(gi == 1 and bb == GBv - 1),
                                      skip_group_check=True, is_transpose=True,
                                  )
                          mx = mxpool.tile([U, 2 * GBv], F32, tag=f"mx{gA}")
                          nc.vector.tensor_reduce(
                              out=mx[:, :],
                              in_=sc[:, :].rearrange("p (b i) -> p b i", i=U),
                              axis=mybir.AxisListType.X, op=mybir.AluOpType.max,
                          )
                          for gi, g in enumerate((gA, gB)):
                              lcols0 = t * BL + g * GBv
                              vcols0 = t * GBv
                              nc.gpsimd.tensor_tensor(
                                  out=vh[g][:, vcols0:vcols0 + GBv],
                                  in0=mx[:, gi * GBv:(gi + 1) * GBv],
                                  in1=logitsT[:, lcols0:lcols0 + GBv],
                                  op=mybir.AluOpType.add,
                              )
                      if (t + 1) % steps_per_chunk == 0:
                          c = (t + 1) // steps_per_chunk - 1
                          for g in range(NGv):
                              nc.sync.dma_start(
                                  out=vout[:, g * (nt // NGv) + c * gch:
                                           g * (nt // NGv) + (c + 1) * gch],
                                  in_=vh[g][:, c * gch:(c + 1) * gch],
                              )
                      continue
                  for g in range(NGv):
                      lcols0 = t * BL + g * GBv   # logitsT columns
                      vcols0 = t * GBv            # vh[g] columns
                      pcol0 = (t - 1) * GBv
                      sc = scpools[g].tile([128, sctile], F32, tag=f"sc{g}",
                                           name=f"scps{g}")
                      if prefire:
                          # trans matmul first: independent of v, fires as
                          # soon as the PSUM bank frees (off the chain)
                          nc.tensor.matmul(
                              out=sc[:, 0:GBv * U], lhsT=trans_r,
                              rhs=identrep_r,
                              start=True, stop=False, skip_group_check=True,
                          )
                      # broadcast v columns: sc[j, (bb, i)] += v_bb[i]
                      nbc = 1 if skip_bcast else GBv  # probe: 1 bcast only
                      for bb in range(nbc):
                          vcol = vh[g][:, pcol0 + bb:pcol0 + bb + 1]
                          nc.tensor.matmul(
                              out=sc[:, bb * U:(bb + 1) * U],
                              lhsT=vcol.broadcast_to([U, U]), rhs=ident_sb[:, :],
                              start=(bb == 0 and not prefire),
                              stop=(prefire and bb == nbc - 1),
                              skip_group_check=True,
                              is_transpose=True,
                          )
                      for _e in range(extra_pe):
                          vcol = vh[g][:, pcol0:pcol0 + 1]
                          nc.tensor.matmul(
                              out=sc[:, 0:U],
                              lhsT=vcol.broadcast_to([U, U]), rhs=ident_sb[:, :],
                              start=False, stop=False, skip_group_check=True,
                              is_transpose=True,
                          )
                      for _w in range(warm):
                          nc.tensor.matmul(
                              out=sc[:, :], lhsT=trans_r, rhs=identrep_r,
                              start=False, stop=False, skip_group_check=True,
                          )
                      if not prefire:
                          # += trans[i, j] for all batches in one fp32r matmul
                          nc.tensor.matmul(
                              out=sc[:, 0:GBv * U], lhsT=trans_r,
                              rhs=identrep_r,
                              start=False, stop=True, skip_group_check=True,
                          )
                      sb = GBv // redsplit          # batches per reduce op
                      mx = mxpool.tile([U, GBv], F32, tag=f"mx{g}")
                      for rs in range(redsplit):
                          nc.vector.tensor_reduce(
                              out=mx[:, rs * sb:(rs + 1) * sb],
                              in_=sc[:, rs * sb * U:rs * sb * U + sb * redw]
                                  .rearrange("p (b i) -> p b i", i=redw),
                              axis=mybir.AxisListType.X, op=mybir.AluOpType.max,
                          )
                          if ladd == "act":
                              for bb in range(rs * sb, (rs + 1) * sb):
                                  nc.scalar.activation(
                                      out=vh[g][:, vcols0 + bb:vcols0 + bb + 1],
                                      in_=mx[:, bb:bb + 1],
                                      func=mybir.ActivationFunctionType.Identity,
                                      bias=logitsT[:, lcols0 + bb:lcols0 + bb + 1],
                                  )
                          elif ladd == "gpsimd" or (ladd == "split" and g < NGv // 2):
                              nc.gpsimd.tensor_tensor(
                                  out=vh[g][:, vcols0 + rs * sb:
                                            vcols0 + (rs + 1) * sb],
                                  in0=mx[:, rs * sb:(rs + 1) * sb],
                                  in1=logitsT[:, lcols0 + rs * sb:
                                              lcols0 + (rs + 1) * sb],
                                  op=mybir.AluOpType.add,
                              )
                          elif ladd:
                              nc.vector.tensor_tensor(
                                  out=vh[g][:, vcols0 + rs * sb:
                                            vcols0 + (rs + 1) * sb],
                                  in0=mx[:, rs * sb:(rs + 1) * sb],
                                  in1=logitsT[:, lcols0 + rs * sb:
                                              lcols0 + (rs + 1) * sb],
                                  op=mybir.AluOpType.add,
                              )
                          else:
                              nc.vector.tensor_copy(
                                  out=vh[g][:, vcols0 + rs * sb:
                                            vcols0 + (rs + 1) * sb],
                                  in_=mx[:, rs * sb:(rs + 1) * sb],
                              )
                  if (t + 1) % steps_per_chunk == 0:
                      c = (t + 1) // steps_per_chunk - 1
                      for g in range(NGv):
                          nc.sync.dma_start(
                              out=vout[:, g * (nt // NGv) + c * gch:
                                       g * (nt // NGv) + (c + 1) * gch],
                              in_=vh[g][:, c * gch:(c + 1) * gch],
                          )
    return split_multi_waits(nc) if split_waits else nc


def make_in_map(x_core, ker, bias, trans, lb, rb, t_steps=T, d_dim=D):
    """x_core: [BL, t_steps, d_dim] float32."""
    nt = t_steps * BL
    xdt = np.ascontiguousarray(x_core.transpose(2, 1, 0)).reshape(d_dim, nt)
    return {
        "xdt": xdt.astype(np.float32),
        "ker": np.ascontiguousarray(ker, dtype=np.float32),
        "biasrow": np.ascontiguousarray(bias, dtype=np.float32).reshape(1, U),
        "onesrow": np.ones((1, 512), dtype=np.float32),
        "translhs": np.ascontiguousarray(trans, dtype=np.float32),
        "ident": np.eye(U, dtype=np.float32),
        "lbv": np.ascontiguousarray(lb, dtype=np.float32).reshape(U, 1),
        "rbv": np.ascontiguousarray(rb, dtype=np.float32).reshape(U, 1),
    }


def backtrace(v, trans):
    """v: [b, t, u] forward max values; trans: [u, u]. Returns int tags [b, t]."""
    nb, nt, nu = v.shape
    tags = np.zeros((nb, nt), dtype=np.int64)
    cur = np.argmax(v[:, -1, :], axis=1)
    tags[:, -1] = cur
    for t in range(nt - 2, -1, -1):
        scores = v[:, t, :] + trans[:, cur].T     # fp32, same as device order
        cur = np.argmax(scores, axis=1)
        tags[:, t] = cur
    return tags


def vout_to_v(vout_core, t_steps=T, ng=None):
    """vout [U, (g, t, bb)] -> v [BL, t, U] with b = g * gb + bb."""
    if ng is None:
        ng = SHIP.get("ng", NG)
    gb = BL // ng
    v = vout_core.reshape(U, ng, t_steps, gb)     # [u, g, t, bb]
    return np.ascontiguousarray(v.transpose(1, 3, 2, 0).reshape(BL, t_steps, U))


def kernel(x, kernel, bias, chain_kernel, left_boundary, right_boundary):
    x = np.asarray(x, dtype=np.float32)
    ker = np.asarray(kernel, dtype=np.float32)
    bias = np.asarray(bias, dtype=np.float32)
    trans = np.asarray(chain_kernel, dtype=np.float32)
    lb = np.asarray(left_boundary, dtype=np.float32)
    rb = np.asarray(right_boundary, dtype=np.float32)

    nc = build_program(**SHIP)
    in_maps = [
        make_in_map(x[c * BL:(c + 1) * BL], ker, bias, trans, lb, rb)
        for c in range(NCORES)
    ]
    kwargs = {}
    if os.environ.get("CRF_TRACE"):
        kwargs = {"trace": True, "tmpdir": os.environ.get("CRF_TRACE_DIR") or None}
    res = run_bass_kernel_spmd(nc, in_maps, core_ids=list(range(NCORES)), **kwargs)
    global last_results
    last_results = res
    v = np.concatenate(
        [vout_to_v(np.asarray(r["vout"])) for r in res.results], axis=0)
    # the transition the device actually added (fp32r-rounded), for a
    # bit-consistent backtrace
    tprime = np.asarray(res.results[0]["tprime"])          # [j, i] = trans'[i, j]
    trans_eff = np.ascontiguousarray(tprime.T)
    tags = backtrace(v, trans_eff)
    return tags.astype(np.float32)
